# revision 58
# baseline (speedup 1.0000x reference)
"""Causal self-attention Trainium2 kernel (V3).

Problem: B=8, T=1024, C=2048, 16 heads x 128 head-dim, fp32, causal.
Sharding: data-parallel over batch -- each of the 8 NeuronCores computes one
batch element end-to-end; no collectives.

Everything stays resident in SBUF (no DRAM spills). fp16 is used for all
matmul operands (same PE rate as bf16/f32r-wide, 3 more mantissa bits;
all tensors here are O(1)-ranged). Weights are converted fp32->fp16 either
in flight by gpsimd software-DGE casting DMAs or via staged hardware-queue
loads + DVE copies. DMA traffic is spread across the SP and ACT hardware
queues plus the gpsimd software queue.

  phase A+B (one pool, interleaved DMA queues): x row tiles -> PE transpose
    (fp32) -> xT (fp16); v = x @ W_v written straight into per-head tiles
    vh[h] laid out [k-part, d] per 128-k-block with a ones column at col 128
    of each VW-wide block.
  phase C per head h (software-pipelined): W_qk casting DMA (head 0 is
    prefetched during A/B); qT/kT[d,T] fp16 via PE + ACT bias-evac;
    score tiles sT[k,q] = kT-block ^T-free @ qT 256-chunk; exps = Exp (ACT,
    fp16), diagonal blocks causal-masked (DVE) with halved exp/mask work;
    av(h-1) interleaved between score tiles: yacc2[q,129] (PSUM) +=
    exps-128-slice ^T-free @ [v|1] (moving 129) so col 128 accumulates the
    softmax denominator; rden = 1/den and y_norm = yacc2 * rden (DVE,
    per-partition scalar); yT via PE transpose (fp16) -> resident yTh[h].
    On the last head, same-head av groups drain as their j-blocks complete.
  phase D: out = y @ W_proj (hh-outer inside 512-wide n-chunks; 8 PSUM
    banks hold [T, 512]); lhsT = yTh slices, rhs = wp (fp16, casting DMAs,
    6 tiles prefetched during the last head); per-i evacuation alternates
    ACT/DVE and out-writes alternate the SP/ACT DMA queues.

b_attn[2C:] / b_proj are zero-filled per the problem spec; kernel() checks
at runtime and only includes the K=1 bias matmuls if they are nonzero.
"""

import math
from contextlib import ExitStack

import numpy as np

import concourse.bass as bass
import concourse.mybir as mybir
import concourse.tile as tile
from concourse.masks import make_identity
from concourse.vector_clock import ScopedClock

F32 = mybir.dt.float32
F32R = mybir.dt.float32r
BF16 = mybir.dt.bfloat16
F8E4 = mybir.dt.float8e4

B, T, C = 8, 1024, 2048
NH, HD = 16, 128
P = 128
TT = T // P            # 8 row tiles
CT = C // P            # 16 channel tiles
QP = 256               # q-pair width for score matmuls
NQP = T // QP          # 4 q-pairs
VW = 130               # per-k-block stride in vh tiles: 128 v cols + ones col
SM_SCALE = 1.0 / math.sqrt(HD)

N_CORES = 8

# --------------------------------------------------------------------------
# Walrus workaround: this container's walrus rejects any instruction with
# more than one sync wait command. Split multi-wait instructions into a
# chain of single-wait NoOps/Drains on the same engine (engine queues
# process waits in order, so semantics are unchanged).
# --------------------------------------------------------------------------
_orig_commit_instruction = tile.TileContext._commit_instruction


def _patched_commit_instruction(self, inst, lazy_reg_writes=True):
    si = inst.sync_info
    if (
        si is not None
        and len(si.on_wait) > 1
        and inst.engine != mybir.EngineType.Unassigned
    ):
        waits = list(si.on_wait)
        for w in waits[:-1]:
            nop = mybir.InstNoOp(
                name=self.nc.get_next_instruction_name(),
                engine=inst.engine,
                bass_nofuse=True,
                sync_info=mybir.SyncInfo(on_wait=[w], on_update=[]),
            )
            _orig_commit_instruction(self, nop, lazy_reg_writes=False)
        inst.sync_info = mybir.SyncInfo(
            on_wait=[waits[-1]], on_update=list(si.on_update)
        )
    return _orig_commit_instruction(self, inst, lazy_reg_writes=lazy_reg_writes)


def _patched_drain_and_barrier(self, tick_clock, wait_clock):
    drain_inst = self.nc.sync.drain()
    wait_clock.add_sem_waits(
        drain_inst.ins, ScopedClock({None: tick_clock.global_clock})
    )
    si = drain_inst.ins.sync_info
    if si is not None and len(si.on_wait) > 1:
        waits = list(si.on_wait)
        drain_inst.ins.sync_info = mybir.SyncInfo(
            on_wait=[waits[0]], on_update=list(si.on_update)
        )
        for w in waits[1:]:
            d2 = self.nc.sync.drain()
            d2.ins.sync_info = mybir.SyncInfo(on_wait=[w], on_update=[])
    self.nc.all_engine_barrier()
    assert self.sems is not None
    popped = self.nc._tile_sem_poison_stack.pop()
    assert popped is self._sem_poison
    self.nc.clear_and_free_semaphores(list(self.sems.allocated().values()))
    self.nc.all_engine_barrier()


def _apply_patches():
    tile.TileContext._commit_instruction = _patched_commit_instruction
    tile.TileContext._drain_and_barrier = _patched_drain_and_barrier


# --------------------------------------------------------------------------
# Kernel builder
# --------------------------------------------------------------------------

def build_kernel(mode: str = "f32r", with_bias: bool = False) -> bass.Bass:
    """mode: 'f32r' (fast path) or 'f32' (full fp32, debugging).
    with_bias: include the b_attn[2C:]/b_proj K=1 bias matmuls. The problem
    spec fills both biases with zeros, so kernel() selects the fast
    bias-free build unless it sees nonzero values at runtime."""
    _apply_patches()
    mm_dt = F32R if mode == "f32r" else F32
    # fp16 over bf16: same PE/DVE throughput, 3 more mantissa bits; every
    # tensor here is O(1)-ranged so fp16's narrower exponent cannot overflow.
    # (fp8e4 attention passed compile but its quantization tail pushed max
    # rel err to 2.9e-2, over the 2e-2 gate.)
    FP16 = mybir.dt.float16
    at_dt = FP16 if mode == "f32r" else F32
    a8_dt = FP16 if mode == "f32r" else F32

    nc = bass.Bass("TRN2", target_bir_lowering=False, debug=False)

    x_ap = nc.dram_tensor("x", [T, C], F32, kind="ExternalInput").ap()
    wa_ap = nc.dram_tensor("W_attn", [C, 3 * C], F32, kind="ExternalInput").ap()
    ba_ap = nc.dram_tensor("b_attn", [3 * C], F32, kind="ExternalInput").ap()
    wp_ap = nc.dram_tensor("W_proj", [C, C], F32, kind="ExternalInput").ap()
    bp_ap = nc.dram_tensor("b_proj", [C], F32, kind="ExternalInput").ap()
    out_ap = nc.dram_tensor("out", [T, C], F32, kind="ExternalOutput").ap()

    def r(ap):
        return ap.bitcast(mm_dt) if mm_dt is F32R else ap

    def _convert(nc, dst, src, eng):
        # fp32 -> bf16 weight conversion on the least-busy engine
        if eng == "act":
            nc.scalar.activation(dst, src,
                                 mybir.ActivationFunctionType.Copy)
        elif eng == "dve":
            with nc.allow_low_precision(reason="bf16 weights are in budget"):
                nc.vector.tensor_copy(dst, src)
        else:
            with nc.allow_low_precision(reason="bf16 weights are in budget"):
                nc.gpsimd.tensor_copy(dst, src)

    # DRAM views
    x_rows = x_ap.rearrange("(i p) c -> i p c", p=P)          # [TT, P, C]
    out_rows = out_ap.rearrange("(i p) c -> i p c", p=P)      # [TT, P, C]
    wa_3d = wa_ap.rearrange("(j p) n -> p j n", p=P)          # [P, CT, 3C]
    wp_rows = wp_ap.rearrange("(h p) n -> h p n", p=P)        # [NH, P, C]
    ba_col = ba_ap.rearrange("(n p one) -> n p one", p=P, one=1)  # [48, P, 1]
    bv_row = ba_ap.rearrange("(n c) -> n c", n=3)             # [3, C]
    bp_row = bp_ap.rearrange("(one c) -> one c", one=1)       # [1, C]

    with tile.TileContext(nc) as tc, ExitStack() as ctx:
        # ---------------- constants ----------------
        const = ctx.enter_context(tc.tile_pool(name="const", bufs=1))
        # identities and masks are built in fp32 (the only dtype the
        # memset/affine_select codegen path supports) and converted
        ident_f = const.tile([P, P], F32)
        make_identity(nc, ident_f[:])
        ident = ident_f[:]
        ident_bt = const.tile([P, P], at_dt)
        with nc.allow_low_precision(reason="exact 0/1 identity"):
            nc.vector.tensor_copy(ident_bt[:], ident_f[:])
        ident_b = ident_bt[:]
        # lower-triangular causal mask for diagonal k-blocks:
        # maskT[k, q] = 1 if q >= k else 0 (both diagonal cases reduce to it)
        maskT_f = const.tile([P, P], F32)
        nc.gpsimd.memset(maskT_f[:], 1.0)
        nc.gpsimd.affine_select(
            out=maskT_f[:], in_=maskT_f[:], compare_op=mybir.AluOpType.is_ge,
            fill=0.0, base=0, pattern=[[1, P]], channel_multiplier=-1)
        maskT = const.tile([P, P], a8_dt)
        with nc.allow_low_precision(reason="exact 0/1 mask"):
            nc.vector.tensor_copy(maskT[:], maskT_f[:])
        # ones row (K=1 bias matmul lhsT)
        ones_row_f = const.tile([1, P], F32)
        nc.vector.memset(ones_row_f[:], 1.0)
        ones_row = const.tile([1, P], mm_dt)
        nc.vector.tensor_copy(ones_row[:], ones_row_f[:])

        # ---------------- resident tensors ----------------
        # yTh spans phases C-D; xT/vh close after attention (work_ctx).
        res_pool = ctx.enter_context(tc.tile_pool(name="resident", bufs=1))
        yTh = [res_pool.tile([P, T], at_dt, tag=f"yTh{h}", name=f"yTh{h}")
               for h in range(NH)]
        # prefetch targets for phase D (W_proj head tiles + bias); the pool
        # must outlive work_ctx, so it is opened here. DMAs for wp_pre are
        # emitted late (during the last attention head).
        pre_pool = ctx.enter_context(tc.tile_pool(name="pre", bufs=1))
        if with_bias:
            bp_sb = pre_pool.tile([1, C], mm_dt, tag="bp")
            nc.sync.dma_start(bp_sb[:], r(bp_row[:, :]))
        N_PRE = 6
        wp_pre = [pre_pool.tile([P, C], at_dt, tag=f"wpp{hh}", name=f"wpp{hh}")
                  for hh in range(N_PRE)]
        # head 0's q/k weights land in the long-lived pre pool so their
        # casting DMAs can run during phase A/B (fresh addresses, no WAR on
        # the attention pool's reused ranges)
        wq0 = pre_pool.tile([P, C], at_dt, tag="wq0")
        wk0 = pre_pool.tile([P, C], at_dt, tag="wk0")
        work_ctx = ExitStack()
        work = work_ctx.enter_context(tc.tile_pool(name="work", bufs=1))
        xT = [work.tile([P, T], at_dt, tag=f"xT{j}", name=f"xT{j}")
              for j in range(CT)]
        # vh[h]: per k-block j, cols [j*VW, j*VW+128) = v rows of block j for
        # head h; col j*VW+128 = 1.0 (softmax denominator); col +129 unused.
        vh = [work.tile([P, TT * VW], a8_dt, tag=f"vh{h}", name=f"vh{h}")
              for h in range(NH)]
        for h in range(NH):
            # split memsets across Pool and DVE so neither gates the start
            if h % 2 == 0:
                nc.gpsimd.memset(vh[h][:], 1.0)
            else:
                nc.vector.memset(vh[h][:], 1.0)

        # ---------------- phase A+B: x -> xT and v = x @ W_v + b_v --------
        # single pool so the x row-tile DMAs and W_v chunk DMAs can be
        # interleaved in the queue: the PE starts v-chunk np as soon as its
        # weights and the transposed row tiles are in, instead of waiting
        # for all of x first.
        NW = 4                           # n-chunks of W_v (512 wide each)
        CW = C // NW
        with tc.tile_pool(name="psA", bufs=1, space="PSUM") as psA, \
             tc.tile_pool(name="phAB", bufs=1) as phAB:
            engs = ["act", "dve", "pool"]
            xas, wv = [], {}

            def emit_xa(i):
                xa = phAB.tile([P, C], F32, tag="xa", bufs=4, name=f"xa{i}")
                if i % 2 == 0:
                    nc.sync.dma_start(xa[:], x_rows[i])
                else:
                    nc.scalar.dma_start(xa[:], x_rows[i])
                xas.append(xa)

            def emit_wv_batch(np_i, c0, c1):
                # two parallel load streams: even c-tiles ride the gpsimd
                # software-DGE with an fp32 -> fp16 cast in flight; odd
                # c-tiles ride the SP hardware queue staged + DVE-converted
                for c in range(c0, c1):
                    wvc = phAB.tile([P, CW], at_dt, tag=f"wv{c}", bufs=2,
                                    name=f"wv{c}_{np_i}")
                    src = wa_3d[:, c,
                                2 * C + np_i * CW: 2 * C + (np_i + 1) * CW]
                    if c % 2 == 0:
                        nc.gpsimd.dma_start(wvc[:], src)
                    else:
                        wvs = phAB.tile([P, CW], F32, tag="wvs", bufs=3,
                                        name=f"wvs{c}_{np_i}")
                        nc.sync.dma_start(wvs[:], src)
                        _convert(nc, wvc[:], wvs[:], "dve")
                    wv[(np_i, c)] = wvc

            emit_xa(0)
            emit_xa(1)
            if with_bias:
                bv_sb = phAB.tile([1, C], mm_dt, tag="bv")
                nc.sync.dma_start(bv_sb[:], r(bv_row[2:3, :]))
            batches = [(np_i, c0, c0 + 8)
                       for np_i in range(NW) for c0 in (0, 8)]
            for i in range(2, TT):
                emit_wv_batch(*batches.pop(0))
                emit_xa(i)
                if i == 3:
                    nc.gpsimd.dma_start(
                        wq0[:].rearrange("p (j f) -> p j f", f=P),
                        wa_3d[:, :, 0:P])
                    nc.gpsimd.dma_start(
                        wk0[:].rearrange("p (j f) -> p j f", f=P),
                        wa_3d[:, :, C:C + P])
            for b in batches:
                emit_wv_batch(*b)

            for i in range(TT):
                xa = xas[i]
                for j in range(CT):
                    tp = psA.tile([P, P], F32, tag="tp", bufs=4)
                    nc.tensor.transpose(tp[:], xa[:, j * P:(j + 1) * P],
                                        ident)
                    # alternate evacuation across DVE/ACT (f32 -> fp16);
                    # Pool cannot read PSUM
                    dst = xT[j][:, i * P:(i + 1) * P]
                    if j % 2 == 0:
                        with nc.allow_low_precision(
                                reason="fp16 x is within budget"):
                            nc.vector.tensor_copy(dst, tp[:])
                    else:
                        nc.scalar.activation(
                            dst, tp[:],
                            mybir.ActivationFunctionType.Copy)

            for np_i in range(NW):
                for i in range(TT):
                    pv = psA.tile([P, CW], F32, tag="pv", bufs=4,
                                  name=f"pv{np_i}_{i}")
                    for c in range(CT):
                        nc.tensor.matmul(
                            pv[:], xT[c][:, i * P:(i + 1) * P],
                            wv[(np_i, c)][:],
                            start=(c == 0),
                            stop=(not with_bias and c == CT - 1))
                    if with_bias:
                        nc.tensor.matmul(
                            pv[:], ones_row[:],
                            bv_sb[:, np_i * CW:(np_i + 1) * CW],
                            start=False, stop=True)
                    # scatter the 4 head-column blocks into vh tiles;
                    # alternate ACT/DVE
                    for hq in range(CW // P):
                        h = np_i * (CW // P) + hq
                        dst = vh[h][:, i * VW: i * VW + P]
                        src = pv[:, hq * P:(hq + 1) * P]
                        if hq % 2 == 0:
                            nc.scalar.activation(
                                dst, src,
                                mybir.ActivationFunctionType.Copy)
                        else:
                            with nc.allow_low_precision(
                                    reason="bf16 v is within budget"):
                                nc.vector.tensor_copy(dst, src)

        # ---------------- phase C: per-head attention (pipelined) ---------
        psB = work_ctx.enter_context(
            tc.tile_pool(name="psB", bufs=1, space="PSUM"))
        att = work_ctx.enter_context(tc.tile_pool(name="att", bufs=2))
        exps_pool = work_ctx.enter_context(tc.tile_pool(name="exps", bufs=28))


        qTs, kTs, exps = {}, {}, {}

        def emit_qk(h):
            if h == 0:
                wq, wk = wq0, wk0
            else:
                wq = att.tile([P, C], at_dt, tag="wq", bufs=2, name=f"wq{h}")
                nc.gpsimd.dma_start(
                    wq[:].rearrange("p (j f) -> p j f", f=P),
                    wa_3d[:, :, h * P:(h + 1) * P])
                wk = att.tile([P, C], at_dt, tag="wk", bufs=2, name=f"wk{h}")
                nc.gpsimd.dma_start(
                    wk[:].rearrange("p (j f) -> p j f", f=P),
                    wa_3d[:, :, C + h * P: C + (h + 1) * P])
            bq = att.tile([P, 1], F32, tag="bq", name=f"bq{h}")
            nc.sync.dma_start(bq[:], ba_col[h])
            bk = att.tile([P, 1], F32, tag="bk", name=f"bk{h}")
            nc.sync.dma_start(bk[:], ba_col[NH + h])

            qT = att.tile([P, T], at_dt, tag="qT", bufs=1, name=f"qT{h}")
            kT = att.tile([P, T], at_dt, tag="kT", bufs=1, name=f"kT{h}")
            qTs[h], kTs[h] = qT, kT
            for di, (dst, w, bias) in enumerate(
                    ((qT, wq, bq), (kT, wk, bk))):
                pq = [psB.tile([P, 512], F32, tag="pq", bufs=4,
                               name=f"pq{h}_{di}_{ch}")
                      for ch in range(T // 512)]
                for c in range(CT):
                    for ch in range(T // 512):
                        nc.tensor.matmul(
                            pq[ch][:], w[:, c * P:(c + 1) * P],
                            xT[c][:, ch * 512:(ch + 1) * 512],
                            start=(c == 0), stop=(c == CT - 1))
                for ch in range(T // 512):
                    nc.scalar.activation(
                        dst[:, ch * 512:(ch + 1) * 512], pq[ch][:],
                        mybir.ActivationFunctionType.Identity,
                        bias=bias[:])

        def score_steps(h):
            # one closure per (j, p_i) score tile. Diagonal blocks get
            # halved exp/mask work:
            #   j == 2p_i:   only the left 128 q-cols need masking
            #   j == 2p_i+1: left 128 q-cols are fully masked -> store a
            #                [P, P] tile of the right half only
            qT, kT = qTs[h], kTs[h]
            steps = []
            for j in range(2 * NQP):
                for p_i in range(j // 2, NQP):
                    def step(j=j, p_i=p_i):
                        qs = slice(p_i * QP, (p_i + 1) * QP)
                        sT = psB.tile([P, QP], F32, tag="sT", bufs=2,
                                      name=f"sT{h}_{j}_{p_i}")
                        nc.tensor.matmul(
                            sT[:], kT[:, j * P:(j + 1) * P], qT[:, qs],
                            start=True, stop=True)
                        if j == 2 * p_i + 1:
                            ex = exps_pool.tile([P, P], a8_dt, tag="exps_h",
                                                name=f"ex{h}_{j}_{p_i}")
                            nc.scalar.activation(
                                ex[:], sT[:, P:QP],
                                mybir.ActivationFunctionType.Exp,
                                scale=SM_SCALE)
                            with nc.allow_low_precision(
                                    reason="exact 0/1 mask"):
                                nc.vector.tensor_mul(ex[:], ex[:], maskT[:])
                        else:
                            ex = exps_pool.tile([P, QP], a8_dt, tag="exps",
                                                name=f"ex{h}_{j}_{p_i}")
                            nc.scalar.activation(
                                ex[:], sT[:],
                                mybir.ActivationFunctionType.Exp,
                                scale=SM_SCALE)
                            if j == 2 * p_i:
                                with nc.allow_low_precision(
                                        reason="exact 0/1 mask"):
                                    nc.vector.tensor_mul(
                                        ex[:, 0:P], ex[:, 0:P], maskT[:])
                        exps[(h, j, p_i)] = ex
                    steps.append(step)
            return steps

        y_norms = {}

        def av_steps(h):
            # one closure per q-block: accumulate yacc2[q, 0:128] = raw y,
            # [:, 128] = softmax denom, then normalize on DVE.
            steps = []
            for qb in range(TT):
                def step(qb=qb):
                    p_i, half = qb // 2, (qb % 2) * P
                    yacc2 = psB.tile([P, P + 1], F32, tag="yacc2", bufs=2,
                                     name=f"yacc2_{h}_{qb}")
                    for j in range(qb + 1):
                        ex = exps[(h, j, p_i)]
                        exs = ex[:, 0:P] if ex.shape[1] == P \
                            else ex[:, half:half + P]
                        nc.tensor.matmul(
                            yacc2[:], exs,
                            vh[h][:, j * VW: j * VW + P + 1],
                            start=(j == 0), stop=(j == qb))
                    rden = att.tile([P, 1], F32, tag="rden", bufs=8,
                                    name=f"rden{h}_{qb}")
                    with nc.allow_low_precision(
                            reason="softmax denom recip is within budget"):
                        nc.vector.reciprocal(rden[:], yacc2[:, P:P + 1])
                    y_norm = att.tile([P, P], at_dt, tag="y_norm", bufs=16,
                                      name=f"yn{h}_{qb}")
                    with nc.allow_low_precision(
                            reason="bf16 y is within budget"):
                        nc.vector.tensor_scalar_mul(
                            y_norm[:], yacc2[:, 0:P], rden[:])
                    y_norms[(h, qb)] = y_norm
                steps.append(step)
            return steps

        def emit_ytp(h):
            # PE transposes of the normalized y blocks -> resident yTh
            for qb in range(TT):
                ytp = psB.tile([P, P], at_dt, tag="sT", bufs=2,
                               name=f"ytp{h}_{qb}")
                nc.tensor.transpose(ytp[:], y_norms[(h, qb)][:], ident_b)
                nc.scalar.activation(
                    yTh[h][:, qb * P:(qb + 1) * P], ytp[:],
                    mybir.ActivationFunctionType.Copy)
                del y_norms[(h, qb)]
            for key in [k for k in exps if k[0] == h]:
                del exps[key]

        def interleave(sc, av, lead=8):
            # spread the (few, chain-heavy) av groups between the (many)
            # score tiles so the PE always has independent matmuls while
            # the ACT engine drains the exp queue. The first `lead` groups
            # go before any score tile: they cover the latency of the
            # qT/kT PSUM evacuation the first score matmul waits on.
            out, ai = [], min(lead, len(av))
            out.extend(av[:ai])
            for si, s_step in enumerate(sc):
                out.append(s_step)
                want = (si + 1) * (len(av) - ai) // len(sc)
                while want > 0:
                    out.append(av[ai])
                    ai += 1
                    want -= 1
            out.extend(av[ai:])
            return out

        # score_steps emits j-outer; av group qb of the same head is ready
        # once all j <= qb tiles of its pair exist. sc_j_done[J] = index into
        # the sc list after which j-block J is fully emitted.
        def sc_count_through(J):
            return sum(NQP - j // 2 for j in range(J + 1))

        for h in range(NH):
            if h == NH - 1:
                # prefetch the first W_proj tiles (casting DMA) while
                # attention drains
                for hh in range(N_PRE):
                    nc.gpsimd.dma_start(wp_pre[hh][:], wp_rows[hh])
            emit_qk(h)
            sc = score_steps(h)
            av = av_steps(h - 1) if h > 0 else []
            if h < NH - 1:
                for step in interleave(sc, av):
                    step()
                if h > 0:
                    emit_ytp(h - 1)
            else:
                # last head: also drain same-head av groups as their score
                # j-blocks complete, so the epilogue isn't serialized on the
                # ACT exp queue
                av15 = av_steps(h)
                done15 = 0
                sc_seen = 0
                for step in interleave(sc, av):
                    step()
                    if step in sc:
                        sc_seen += 1
                        while (done15 < TT
                               and sc_seen >= sc_count_through(done15)):
                            av15[done15]()
                            done15 += 1
                for qb in range(done15, TT):
                    av15[qb]()
                emit_ytp(h - 1)
                emit_ytp(h)
        work_ctx.close()

        # ---------------- phase D: out = y @ W_proj + b -------------------
        # hh-outer inside 512-wide n-chunks; 8 PSUM banks hold [T, 512].
        with tc.tile_pool(name="psC", bufs=1, space="PSUM") as psC, \
             tc.tile_pool(name="ph3", bufs=1) as ph3:
            wp = list(wp_pre)
            for hh in range(len(wp_pre), NH):
                wpc = ph3.tile([P, C], at_dt, tag=f"wp{hh}", bufs=1,
                               name=f"wp{hh}")
                nc.gpsimd.dma_start(wpc[:], wp_rows[hh])
                wp.append(wpc)
            for nn in range(4):
                ns = slice(nn * 512, (nn + 1) * 512)
                po = [psC.tile([P, 512], F32, tag=f"po{i}", bufs=1,
                               name=f"po{nn}_{i}")
                      for i in range(TT)]
                for hh in range(NH):
                    for i in range(TT):
                        nc.tensor.matmul(
                            po[i][:], yTh[hh][:, i * P:(i + 1) * P],
                            wp[hh][:, ns],
                            start=(hh == 0),
                            stop=(not with_bias and hh == NH - 1))
                for i in range(TT):
                    if with_bias:
                        nc.tensor.matmul(
                            po[i][:], ones_row[:], bp_sb[:, ns],
                            start=False, stop=True)
                    osb = ph3.tile([P, 512], F32, tag="osb", bufs=8,
                                   name=f"osb{nn}_{i}")
                    if i % 2 == 1:
                        nc.vector.tensor_copy(osb[:], po[i][:])
                    else:
                        nc.scalar.activation(
                            osb[:], po[i][:],
                            mybir.ActivationFunctionType.Copy)
                    if i % 2 == 0:
                        nc.sync.dma_start(out_rows[i][:, ns], osb[:])
                    else:
                        nc.scalar.dma_start(out_rows[i][:, ns], osb[:])

    return nc


_BUILT = {}


def _get_nc(mode: str, with_bias: bool = False):
    key = (mode, with_bias)
    if key not in _BUILT:
        _BUILT[key] = build_kernel(mode, with_bias=with_bias)
    return _BUILT[key]


def kernel(x, W_attn, b_attn, W_proj, b_proj, mode: str = "f32r", **run_kwargs):
    from concourse.bass_utils import run_bass_kernel_spmd

    x = np.asarray(x, dtype=np.float32)
    W_attn = np.ascontiguousarray(np.asarray(W_attn, dtype=np.float32))
    b_attn = np.ascontiguousarray(np.asarray(b_attn, dtype=np.float32))
    W_proj = np.ascontiguousarray(np.asarray(W_proj, dtype=np.float32))
    b_proj = np.ascontiguousarray(np.asarray(b_proj, dtype=np.float32))

    with_bias = bool(np.any(b_attn[2 * C:])) or bool(np.any(b_proj))
    nc = _get_nc(mode, with_bias)
    in_maps = [
        {
            "x": np.ascontiguousarray(x[b]),
            "W_attn": W_attn,
            "b_attn": b_attn,
            "W_proj": W_proj,
            "b_proj": b_proj,
        }
        for b in range(N_CORES)
    ]
    res = run_bass_kernel_spmd(nc, in_maps, list(range(N_CORES)), **run_kwargs)
    out = np.stack([res.results[b]["out"] for b in range(N_CORES)], axis=0)
    kernel.last_results = res
    return out


# revision 59
# speedup vs baseline: 1.0299x; 1.0299x over previous
"""Causal self-attention Trainium2 kernel (V3).

Problem: B=8, T=1024, C=2048, 16 heads x 128 head-dim, fp32, causal.
Sharding: data-parallel over batch -- each of the 8 NeuronCores computes one
batch element end-to-end; no collectives.

Everything stays resident in SBUF (no DRAM spills). fp16 is used for all
matmul operands (same PE rate as bf16/f32r-wide, 3 more mantissa bits;
all tensors here are O(1)-ranged). Weights are converted fp32->fp16 either
in flight by gpsimd software-DGE casting DMAs or via staged hardware-queue
loads + DVE copies. DMA traffic is spread across the SP and ACT hardware
queues plus the gpsimd software queue.

  phase A+B (one pool, interleaved DMA queues): x row tiles -> PE transpose
    (fp32) -> xT (fp16); v = x @ W_v written straight into per-head tiles
    vh[h] laid out [k-part, d] per 128-k-block with a ones column at col 128
    of each VW-wide block.
  phase C per head h (software-pipelined): W_qk casting DMA (head 0 is
    prefetched during A/B); qT/kT[d,T] fp16 via PE + ACT bias-evac;
    score tiles sT[k,q] = kT-block ^T-free @ qT 256-chunk; exps = Exp (ACT,
    fp16), diagonal blocks causal-masked (DVE) with halved exp/mask work;
    av(h-1) interleaved between score tiles: yacc2[q,129] (PSUM) +=
    exps-128-slice ^T-free @ [v|1] (moving 129) so col 128 accumulates the
    softmax denominator; rden = 1/den and y_norm = yacc2 * rden (DVE,
    per-partition scalar); yT via PE transpose (fp16) -> resident yTh[h].
    On the last head, same-head av groups drain as their j-blocks complete.
  phase D: out = y @ W_proj (hh-outer inside 512-wide n-chunks; 8 PSUM
    banks hold [T, 512]); lhsT = yTh slices, rhs = wp (fp16, casting DMAs,
    6 tiles prefetched during the last head); per-i evacuation alternates
    ACT/DVE and out-writes alternate the SP/ACT DMA queues.

b_attn[2C:] / b_proj are zero-filled per the problem spec; kernel() checks
at runtime and only includes the K=1 bias matmuls if they are nonzero.
"""

import math
from contextlib import ExitStack

import numpy as np

import concourse.bass as bass
import concourse.mybir as mybir
import concourse.tile as tile
from concourse.masks import make_identity
from concourse.vector_clock import ScopedClock

F32 = mybir.dt.float32
F32R = mybir.dt.float32r
BF16 = mybir.dt.bfloat16
F8E4 = mybir.dt.float8e4

B, T, C = 8, 1024, 2048
NH, HD = 16, 128
P = 128
TT = T // P            # 8 row tiles
CT = C // P            # 16 channel tiles
QP = 256               # q-pair width for score matmuls
NQP = T // QP          # 4 q-pairs
VW = 130               # per-k-block stride in vh tiles: 128 v cols + ones col
SM_SCALE = 1.0 / math.sqrt(HD)

N_CORES = 8

# --------------------------------------------------------------------------
# Walrus workaround: this container's walrus rejects any instruction with
# more than one sync wait command. Split multi-wait instructions into a
# chain of single-wait NoOps/Drains on the same engine (engine queues
# process waits in order, so semantics are unchanged).
# --------------------------------------------------------------------------
_orig_commit_instruction = tile.TileContext._commit_instruction


def _patched_commit_instruction(self, inst, lazy_reg_writes=True):
    si = inst.sync_info
    if (
        si is not None
        and len(si.on_wait) > 1
        and inst.engine != mybir.EngineType.Unassigned
    ):
        waits = list(si.on_wait)
        for w in waits[:-1]:
            nop = mybir.InstNoOp(
                name=self.nc.get_next_instruction_name(),
                engine=inst.engine,
                bass_nofuse=True,
                sync_info=mybir.SyncInfo(on_wait=[w], on_update=[]),
            )
            _orig_commit_instruction(self, nop, lazy_reg_writes=False)
        inst.sync_info = mybir.SyncInfo(
            on_wait=[waits[-1]], on_update=list(si.on_update)
        )
    return _orig_commit_instruction(self, inst, lazy_reg_writes=lazy_reg_writes)


def _patched_drain_and_barrier(self, tick_clock, wait_clock):
    drain_inst = self.nc.sync.drain()
    wait_clock.add_sem_waits(
        drain_inst.ins, ScopedClock({None: tick_clock.global_clock})
    )
    si = drain_inst.ins.sync_info
    if si is not None and len(si.on_wait) > 1:
        waits = list(si.on_wait)
        drain_inst.ins.sync_info = mybir.SyncInfo(
            on_wait=[waits[0]], on_update=list(si.on_update)
        )
        for w in waits[1:]:
            d2 = self.nc.sync.drain()
            d2.ins.sync_info = mybir.SyncInfo(on_wait=[w], on_update=[])
    self.nc.all_engine_barrier()
    assert self.sems is not None
    popped = self.nc._tile_sem_poison_stack.pop()
    assert popped is self._sem_poison
    self.nc.clear_and_free_semaphores(list(self.sems.allocated().values()))
    self.nc.all_engine_barrier()


def _apply_patches():
    tile.TileContext._commit_instruction = _patched_commit_instruction
    tile.TileContext._drain_and_barrier = _patched_drain_and_barrier


# --------------------------------------------------------------------------
# Kernel builder
# --------------------------------------------------------------------------

def build_kernel(mode: str = "f32r", with_bias: bool = False) -> bass.Bass:
    """mode: 'f32r' (fast path) or 'f32' (full fp32, debugging).
    with_bias: include the b_attn[2C:]/b_proj K=1 bias matmuls. The problem
    spec fills both biases with zeros, so kernel() selects the fast
    bias-free build unless it sees nonzero values at runtime."""
    _apply_patches()
    mm_dt = F32R if mode == "f32r" else F32
    # fp16 over bf16: same PE/DVE throughput, 3 more mantissa bits; every
    # tensor here is O(1)-ranged so fp16's narrower exponent cannot overflow.
    # (fp8e4 attention passed compile but its quantization tail pushed max
    # rel err to 2.9e-2, over the 2e-2 gate.)
    FP16 = mybir.dt.float16
    at_dt = FP16 if mode == "f32r" else F32
    a8_dt = FP16 if mode == "f32r" else F32

    nc = bass.Bass("TRN2", target_bir_lowering=False, debug=False)

    x_ap = nc.dram_tensor("x", [T, C], F32, kind="ExternalInput").ap()
    wa_ap = nc.dram_tensor("W_attn", [C, 3 * C], F32, kind="ExternalInput").ap()
    ba_ap = nc.dram_tensor("b_attn", [3 * C], F32, kind="ExternalInput").ap()
    wp_ap = nc.dram_tensor("W_proj", [C, C], F32, kind="ExternalInput").ap()
    bp_ap = nc.dram_tensor("b_proj", [C], F32, kind="ExternalInput").ap()
    out_ap = nc.dram_tensor("out", [T, C], F32, kind="ExternalOutput").ap()

    def r(ap):
        return ap.bitcast(mm_dt) if mm_dt is F32R else ap

    def _convert(nc, dst, src, eng):
        # fp32 -> bf16 weight conversion on the least-busy engine
        if eng == "act":
            nc.scalar.activation(dst, src,
                                 mybir.ActivationFunctionType.Copy)
        elif eng == "dve":
            with nc.allow_low_precision(reason="bf16 weights are in budget"):
                nc.vector.tensor_copy(dst, src)
        else:
            with nc.allow_low_precision(reason="bf16 weights are in budget"):
                nc.gpsimd.tensor_copy(dst, src)

    # DRAM views
    x_rows = x_ap.rearrange("(i p) c -> i p c", p=P)          # [TT, P, C]
    out_rows = out_ap.rearrange("(i p) c -> i p c", p=P)      # [TT, P, C]
    wa_3d = wa_ap.rearrange("(j p) n -> p j n", p=P)          # [P, CT, 3C]
    wp_rows = wp_ap.rearrange("(h p) n -> h p n", p=P)        # [NH, P, C]
    ba_col = ba_ap.rearrange("(n p one) -> n p one", p=P, one=1)  # [48, P, 1]
    bv_row = ba_ap.rearrange("(n c) -> n c", n=3)             # [3, C]
    bp_row = bp_ap.rearrange("(one c) -> one c", one=1)       # [1, C]

    with tile.TileContext(nc) as tc, ExitStack() as ctx:
        # ---------------- constants ----------------
        const = ctx.enter_context(tc.tile_pool(name="const", bufs=1))
        # identities and masks are built in fp32 (the only dtype the
        # memset/affine_select codegen path supports) and converted
        ident_f = const.tile([P, P], F32)
        make_identity(nc, ident_f[:])
        ident = ident_f[:]
        ident_bt = const.tile([P, P], at_dt)
        with nc.allow_low_precision(reason="exact 0/1 identity"):
            nc.vector.tensor_copy(ident_bt[:], ident_f[:])
        ident_b = ident_bt[:]
        # lower-triangular causal mask for diagonal k-blocks:
        # maskT[k, q] = 1 if q >= k else 0 (both diagonal cases reduce to it)
        maskT_f = const.tile([P, P], F32)
        nc.gpsimd.memset(maskT_f[:], 1.0)
        nc.gpsimd.affine_select(
            out=maskT_f[:], in_=maskT_f[:], compare_op=mybir.AluOpType.is_ge,
            fill=0.0, base=0, pattern=[[1, P]], channel_multiplier=-1)
        maskT = const.tile([P, P], a8_dt)
        with nc.allow_low_precision(reason="exact 0/1 mask"):
            nc.vector.tensor_copy(maskT[:], maskT_f[:])
        # ones row (K=1 bias matmul lhsT)
        ones_row_f = const.tile([1, P], F32)
        nc.vector.memset(ones_row_f[:], 1.0)
        ones_row = const.tile([1, P], mm_dt)
        nc.vector.tensor_copy(ones_row[:], ones_row_f[:])

        # ---------------- resident tensors ----------------
        # yTh spans phases C-D; xT/vh close after attention (work_ctx).
        res_pool = ctx.enter_context(tc.tile_pool(name="resident", bufs=1))
        yTh = [res_pool.tile([P, T], at_dt, tag=f"yTh{h}", name=f"yTh{h}")
               for h in range(NH)]
        # prefetch targets for phase D (W_proj head tiles + bias); the pool
        # must outlive work_ctx, so it is opened here. DMAs for wp_pre are
        # emitted late (during the last attention head).
        pre_pool = ctx.enter_context(tc.tile_pool(name="pre", bufs=1))
        if with_bias:
            bp_sb = pre_pool.tile([1, C], mm_dt, tag="bp")
            nc.sync.dma_start(bp_sb[:], r(bp_row[:, :]))
        N_PRE = 6
        wp_pre = [pre_pool.tile([P, C], at_dt, tag=f"wpp{hh}", name=f"wpp{hh}")
                  for hh in range(N_PRE)]
        # head 0's q/k weights land in the long-lived pre pool so their
        # casting DMAs can run during phase A/B (fresh addresses, no WAR on
        # the attention pool's reused ranges)
        wq0 = pre_pool.tile([P, C], at_dt, tag="wq0")
        wk0 = pre_pool.tile([P, C], at_dt, tag="wk0")
        work_ctx = ExitStack()
        work = work_ctx.enter_context(tc.tile_pool(name="work", bufs=1))
        xT = [work.tile([P, T], at_dt, tag=f"xT{j}", name=f"xT{j}")
              for j in range(CT)]
        # vh[h]: per k-block j, cols [j*VW, j*VW+128) = v rows of block j for
        # head h; col j*VW+128 = 1.0 (softmax denominator); col +129 unused.
        vh = [work.tile([P, TT * VW], a8_dt, tag=f"vh{h}", name=f"vh{h}")
              for h in range(NH)]
        for h in range(NH):
            # split memsets across Pool and DVE so neither gates the start
            if h % 2 == 0:
                nc.gpsimd.memset(vh[h][:], 1.0)
            else:
                nc.vector.memset(vh[h][:], 1.0)

        # ---------------- phase A+B: x -> xT and v = x @ W_v + b_v --------
        # single pool so the x row-tile DMAs and W_v chunk DMAs can be
        # interleaved in the queue: the PE starts v-chunk np as soon as its
        # weights and the transposed row tiles are in, instead of waiting
        # for all of x first.
        NW = 8                           # n-chunks of W_v (256 wide each)
        CW = C // NW
        with tc.tile_pool(name="psA", bufs=1, space="PSUM") as psA, \
             tc.tile_pool(name="phAB", bufs=1) as phAB:
            engs = ["act", "dve", "pool"]
            xas, wv = [], {}

            def emit_xa(i):
                xa = phAB.tile([P, C], F32, tag="xa", bufs=4, name=f"xa{i}")
                if i % 2 == 0:
                    nc.sync.dma_start(xa[:], x_rows[i])
                else:
                    nc.scalar.dma_start(xa[:], x_rows[i])
                xas.append(xa)

            def emit_wv_batch(np_i, c0, c1):
                # two parallel load streams: even c-tiles ride the gpsimd
                # software-DGE with an fp32 -> fp16 cast in flight; odd
                # c-tiles ride the SP hardware queue staged + DVE-converted
                for c in range(c0, c1):
                    wvc = phAB.tile([P, CW], at_dt, tag=f"wv{c}", bufs=2,
                                    name=f"wv{c}_{np_i}")
                    src = wa_3d[:, c,
                                2 * C + np_i * CW: 2 * C + (np_i + 1) * CW]
                    if c % 2 == 0:
                        nc.gpsimd.dma_start(wvc[:], src)
                    else:
                        wvs = phAB.tile([P, CW], F32, tag="wvs", bufs=3,
                                        name=f"wvs{c}_{np_i}")
                        nc.sync.dma_start(wvs[:], src)
                        _convert(nc, wvc[:], wvs[:], "dve")
                    wv[(np_i, c)] = wvc

            emit_xa(0)
            emit_xa(1)
            if with_bias:
                bv_sb = phAB.tile([1, C], mm_dt, tag="bv")
                nc.sync.dma_start(bv_sb[:], r(bv_row[2:3, :]))
            batches = [(np_i, c0, c0 + 8)
                       for np_i in range(NW) for c0 in (0, 8)]
            for i in range(2, TT):
                emit_wv_batch(*batches.pop(0))
                emit_xa(i)
                if i == 3:
                    nc.gpsimd.dma_start(
                        wq0[:].rearrange("p (j f) -> p j f", f=P),
                        wa_3d[:, :, 0:P])
                    nc.gpsimd.dma_start(
                        wk0[:].rearrange("p (j f) -> p j f", f=P),
                        wa_3d[:, :, C:C + P])
            for b in batches:
                emit_wv_batch(*b)

            for i in range(TT):
                xa = xas[i]
                for j in range(CT):
                    tp = psA.tile([P, P], F32, tag="tp", bufs=4)
                    nc.tensor.transpose(tp[:], xa[:, j * P:(j + 1) * P],
                                        ident)
                    # alternate evacuation across DVE/ACT (f32 -> fp16);
                    # Pool cannot read PSUM
                    dst = xT[j][:, i * P:(i + 1) * P]
                    if j % 2 == 0:
                        with nc.allow_low_precision(
                                reason="fp16 x is within budget"):
                            nc.vector.tensor_copy(dst, tp[:])
                    else:
                        nc.scalar.activation(
                            dst, tp[:],
                            mybir.ActivationFunctionType.Copy)

            for np_i in range(NW):
                for i in range(TT):
                    pv = psA.tile([P, CW], F32, tag="pv", bufs=4,
                                  name=f"pv{np_i}_{i}")
                    for c in range(CT):
                        nc.tensor.matmul(
                            pv[:], xT[c][:, i * P:(i + 1) * P],
                            wv[(np_i, c)][:],
                            start=(c == 0),
                            stop=(not with_bias and c == CT - 1))
                    if with_bias:
                        nc.tensor.matmul(
                            pv[:], ones_row[:],
                            bv_sb[:, np_i * CW:(np_i + 1) * CW],
                            start=False, stop=True)
                    # scatter the 4 head-column blocks into vh tiles;
                    # alternate ACT/DVE
                    for hq in range(CW // P):
                        h = np_i * (CW // P) + hq
                        dst = vh[h][:, i * VW: i * VW + P]
                        src = pv[:, hq * P:(hq + 1) * P]
                        if hq % 2 == 0:
                            nc.scalar.activation(
                                dst, src,
                                mybir.ActivationFunctionType.Copy)
                        else:
                            with nc.allow_low_precision(
                                    reason="bf16 v is within budget"):
                                nc.vector.tensor_copy(dst, src)

        # ---------------- phase C: per-head attention (pipelined) ---------
        psB = work_ctx.enter_context(
            tc.tile_pool(name="psB", bufs=1, space="PSUM"))
        att = work_ctx.enter_context(tc.tile_pool(name="att", bufs=2))
        exps_pool = work_ctx.enter_context(tc.tile_pool(name="exps", bufs=28))


        qTs, kTs, exps = {}, {}, {}

        def emit_qk(h):
            if h == 0:
                wq, wk = wq0, wk0
            else:
                wq = att.tile([P, C], at_dt, tag="wq", bufs=2, name=f"wq{h}")
                nc.gpsimd.dma_start(
                    wq[:].rearrange("p (j f) -> p j f", f=P),
                    wa_3d[:, :, h * P:(h + 1) * P])
                wk = att.tile([P, C], at_dt, tag="wk", bufs=2, name=f"wk{h}")
                nc.gpsimd.dma_start(
                    wk[:].rearrange("p (j f) -> p j f", f=P),
                    wa_3d[:, :, C + h * P: C + (h + 1) * P])
            bq = att.tile([P, 1], F32, tag="bq", name=f"bq{h}")
            nc.sync.dma_start(bq[:], ba_col[h])
            bk = att.tile([P, 1], F32, tag="bk", name=f"bk{h}")
            nc.sync.dma_start(bk[:], ba_col[NH + h])

            qT = att.tile([P, T], at_dt, tag="qT", bufs=1, name=f"qT{h}")
            kT = att.tile([P, T], at_dt, tag="kT", bufs=1, name=f"kT{h}")
            qTs[h], kTs[h] = qT, kT
            for di, (dst, w, bias) in enumerate(
                    ((qT, wq, bq), (kT, wk, bk))):
                pq = [psB.tile([P, 512], F32, tag="pq", bufs=3,
                               name=f"pq{h}_{di}_{ch}")
                      for ch in range(T // 512)]
                for c in range(CT):
                    for ch in range(T // 512):
                        nc.tensor.matmul(
                            pq[ch][:], w[:, c * P:(c + 1) * P],
                            xT[c][:, ch * 512:(ch + 1) * 512],
                            start=(c == 0), stop=(c == CT - 1))
                for ch in range(T // 512):
                    nc.scalar.activation(
                        dst[:, ch * 512:(ch + 1) * 512], pq[ch][:],
                        mybir.ActivationFunctionType.Identity,
                        bias=bias[:])

        def score_steps(h):
            # one closure per (j, p_i) score tile. Diagonal blocks get
            # halved exp/mask work:
            #   j == 2p_i:   only the left 128 q-cols need masking
            #   j == 2p_i+1: left 128 q-cols are fully masked -> store a
            #                [P, P] tile of the right half only
            qT, kT = qTs[h], kTs[h]
            steps = []
            for j in range(2 * NQP):
                for p_i in range(j // 2, NQP):
                    def step(j=j, p_i=p_i):
                        qs = slice(p_i * QP, (p_i + 1) * QP)
                        sT = psB.tile([P, QP], F32, tag="sT", bufs=3,
                                      name=f"sT{h}_{j}_{p_i}")
                        nc.tensor.matmul(
                            sT[:], kT[:, j * P:(j + 1) * P], qT[:, qs],
                            start=True, stop=True)
                        if j == 2 * p_i + 1:
                            ex = exps_pool.tile([P, P], a8_dt, tag="exps_h",
                                                name=f"ex{h}_{j}_{p_i}")
                            nc.scalar.activation(
                                ex[:], sT[:, P:QP],
                                mybir.ActivationFunctionType.Exp,
                                scale=SM_SCALE)
                            with nc.allow_low_precision(
                                    reason="exact 0/1 mask"):
                                nc.vector.tensor_mul(ex[:], ex[:], maskT[:])
                        else:
                            ex = exps_pool.tile([P, QP], a8_dt, tag="exps",
                                                name=f"ex{h}_{j}_{p_i}")
                            nc.scalar.activation(
                                ex[:], sT[:],
                                mybir.ActivationFunctionType.Exp,
                                scale=SM_SCALE)
                            if j == 2 * p_i:
                                with nc.allow_low_precision(
                                        reason="exact 0/1 mask"):
                                    nc.vector.tensor_mul(
                                        ex[:, 0:P], ex[:, 0:P], maskT[:])
                        exps[(h, j, p_i)] = ex
                    steps.append(step)
            return steps

        y_norms = {}

        def av_steps(h):
            # one closure per q-block: accumulate yacc2[q, 0:128] = raw y,
            # [:, 128] = softmax denom, then normalize on DVE.
            steps = []
            for qb in range(TT):
                def step(qb=qb):
                    p_i, half = qb // 2, (qb % 2) * P
                    yacc2 = psB.tile([P, P + 1], F32, tag="yacc2", bufs=2,
                                     name=f"yacc2_{h}_{qb}")
                    for j in range(qb + 1):
                        ex = exps[(h, j, p_i)]
                        exs = ex[:, 0:P] if ex.shape[1] == P \
                            else ex[:, half:half + P]
                        nc.tensor.matmul(
                            yacc2[:], exs,
                            vh[h][:, j * VW: j * VW + P + 1],
                            start=(j == 0), stop=(j == qb))
                    rden = att.tile([P, 1], F32, tag="rden", bufs=8,
                                    name=f"rden{h}_{qb}")
                    with nc.allow_low_precision(
                            reason="softmax denom recip is within budget"):
                        nc.vector.reciprocal(rden[:], yacc2[:, P:P + 1])
                    y_norm = att.tile([P, P], at_dt, tag="y_norm", bufs=16,
                                      name=f"yn{h}_{qb}")
                    with nc.allow_low_precision(
                            reason="bf16 y is within budget"):
                        nc.vector.tensor_scalar_mul(
                            y_norm[:], yacc2[:, 0:P], rden[:])
                    y_norms[(h, qb)] = y_norm
                steps.append(step)
            return steps

        def emit_ytp(h):
            # PE transposes of the normalized y blocks -> resident yTh
            for qb in range(TT):
                ytp = psB.tile([P, P], at_dt, tag="sT", bufs=3,
                               name=f"ytp{h}_{qb}")
                nc.tensor.transpose(ytp[:], y_norms[(h, qb)][:], ident_b)
                nc.scalar.activation(
                    yTh[h][:, qb * P:(qb + 1) * P], ytp[:],
                    mybir.ActivationFunctionType.Copy)
                del y_norms[(h, qb)]
            for key in [k for k in exps if k[0] == h]:
                del exps[key]

        def interleave(sc, av, lead=8):
            # spread the (few, chain-heavy) av groups between the (many)
            # score tiles so the PE always has independent matmuls while
            # the ACT engine drains the exp queue. The first `lead` groups
            # go before any score tile: they cover the latency of the
            # qT/kT PSUM evacuation the first score matmul waits on.
            out, ai = [], min(lead, len(av))
            out.extend(av[:ai])
            for si, s_step in enumerate(sc):
                out.append(s_step)
                want = (si + 1) * (len(av) - ai) // len(sc)
                while want > 0:
                    out.append(av[ai])
                    ai += 1
                    want -= 1
            out.extend(av[ai:])
            return out

        # score_steps emits j-outer; av group qb of the same head is ready
        # once all j <= qb tiles of its pair exist. sc_j_done[J] = index into
        # the sc list after which j-block J is fully emitted.
        def sc_count_through(J):
            return sum(NQP - j // 2 for j in range(J + 1))

        for h in range(NH):
            if h == NH - 1:
                # prefetch the first W_proj tiles (casting DMA) while
                # attention drains
                for hh in range(N_PRE):
                    nc.gpsimd.dma_start(wp_pre[hh][:], wp_rows[hh])
            emit_qk(h)
            sc = score_steps(h)
            av = av_steps(h - 1) if h > 0 else []
            if h < NH - 1:
                for step in interleave(sc, av):
                    step()
                if h > 0:
                    emit_ytp(h - 1)
            else:
                # last head: also drain same-head av groups as their score
                # j-blocks complete, so the epilogue isn't serialized on the
                # ACT exp queue
                av15 = av_steps(h)
                done15 = 0
                sc_seen = 0
                for step in interleave(sc, av):
                    step()
                    if step in sc:
                        sc_seen += 1
                        while (done15 < TT
                               and sc_seen >= sc_count_through(done15)):
                            av15[done15]()
                            done15 += 1
                for qb in range(done15, TT):
                    av15[qb]()
                emit_ytp(h - 1)
                emit_ytp(h)
        work_ctx.close()

        # ---------------- phase D: out = y @ W_proj + b -------------------
        # hh-outer inside 512-wide n-chunks; 8 PSUM banks hold [T, 512].
        with tc.tile_pool(name="psC", bufs=1, space="PSUM") as psC, \
             tc.tile_pool(name="ph3", bufs=1) as ph3:
            wp = list(wp_pre)
            for hh in range(len(wp_pre), NH):
                wpc = ph3.tile([P, C], at_dt, tag=f"wp{hh}", bufs=1,
                               name=f"wp{hh}")
                nc.gpsimd.dma_start(wpc[:], wp_rows[hh])
                wp.append(wpc)
            for nn in range(4):
                ns = slice(nn * 512, (nn + 1) * 512)
                po = [psC.tile([P, 512], F32, tag=f"po{i}", bufs=1,
                               name=f"po{nn}_{i}")
                      for i in range(TT)]
                for hh in range(NH):
                    for i in range(TT):
                        nc.tensor.matmul(
                            po[i][:], yTh[hh][:, i * P:(i + 1) * P],
                            wp[hh][:, ns],
                            start=(hh == 0),
                            stop=(not with_bias and hh == NH - 1))
                for i in range(TT):
                    if with_bias:
                        nc.tensor.matmul(
                            po[i][:], ones_row[:], bp_sb[:, ns],
                            start=False, stop=True)
                    osb = ph3.tile([P, 512], F32, tag="osb", bufs=8,
                                   name=f"osb{nn}_{i}")
                    if i % 2 == 1:
                        nc.vector.tensor_copy(osb[:], po[i][:])
                    else:
                        nc.scalar.activation(
                            osb[:], po[i][:],
                            mybir.ActivationFunctionType.Copy)
                    if i % 2 == 0:
                        nc.sync.dma_start(out_rows[i][:, ns], osb[:])
                    else:
                        nc.scalar.dma_start(out_rows[i][:, ns], osb[:])

    return nc


_BUILT = {}


def _get_nc(mode: str, with_bias: bool = False):
    key = (mode, with_bias)
    if key not in _BUILT:
        _BUILT[key] = build_kernel(mode, with_bias=with_bias)
    return _BUILT[key]


def kernel(x, W_attn, b_attn, W_proj, b_proj, mode: str = "f32r", **run_kwargs):
    from concourse.bass_utils import run_bass_kernel_spmd

    x = np.asarray(x, dtype=np.float32)
    W_attn = np.ascontiguousarray(np.asarray(W_attn, dtype=np.float32))
    b_attn = np.ascontiguousarray(np.asarray(b_attn, dtype=np.float32))
    W_proj = np.ascontiguousarray(np.asarray(W_proj, dtype=np.float32))
    b_proj = np.ascontiguousarray(np.asarray(b_proj, dtype=np.float32))

    with_bias = bool(np.any(b_attn[2 * C:])) or bool(np.any(b_proj))
    nc = _get_nc(mode, with_bias)
    in_maps = [
        {
            "x": np.ascontiguousarray(x[b]),
            "W_attn": W_attn,
            "b_attn": b_attn,
            "W_proj": W_proj,
            "b_proj": b_proj,
        }
        for b in range(N_CORES)
    ]
    res = run_bass_kernel_spmd(nc, in_maps, list(range(N_CORES)), **run_kwargs)
    out = np.stack([res.results[b]["out"] for b in range(N_CORES)], axis=0)
    kernel.last_results = res
    return out
